# revision 3
# baseline (speedup 1.0000x reference)
"""Trainium2 Bass kernel for nn_Decoder_Model_EBV (gnn_message_passing).

Math: score[e] = <X_trans[src_e] - X_trans[tgt_e], ebvecs[type_e]>
      with X_trans = X_embed @ W.T.

Folding W into the basis vectors: U = ebvecs @ W  (500 x 512), and
Z = X_embed @ U.T  (100000 x 500) gives
      score[e] = Z[src_e, type_e] - Z[tgt_e, type_e].

Sharding: nodes are split evenly across the 8 NeuronCores (12500 each).
The host pre-transposes each core's X slice to embed-major fp16 layout and
pre-folds U (tiny 500x256x512 matmul) so the device runs ONLY the dense
Z = X @ U.T matmul in fp16 at 1 cycle/row on the PE:
  out chunk [128 nodes, 500 types] accumulates 4 K=128 passes in PSUM.
Input groups ramp 128/256/512 nodes before settling at 1024 so the first
matmul issues as soon as possible after the framework preamble.  Z streams
back to HBM as fp16 in two-chunk tiles; the host gathers the two endpoints
per edge (vertex-cut, zero cross-device communication) and combines.
"""

import numpy as np

import concourse.bass as bass
import concourse.bacc as bacc
import concourse.tile as tile
import concourse.mybir as mybir
from concourse.bass_utils import run_bass_kernel_spmd

# problem constants (hardcoded per spec)
N_NODES = 100000
EMBED = 512
NREL = 500
E = 300000

NCORES = 8
NPC = N_NODES // NCORES          # 12500 nodes per core
P = 128
NEC = EMBED // P                 # 4 contraction chunks
# node-group sizes (all multiples of 128): small head groups let the first
# matmul start right after the framework preamble; 384-node tail pads 12500
# to 12544 = 98 chunks
GROUPS = [128, 256, 512] + [1024] * 11 + [384]
NPAD = sum(GROUPS)               # 12544
NCHUNK = NPAD // P               # 98
NPAIR = NCHUNK // 2 + 1          # 50 output rows (chunk 0 and 97 ride alone)

_compiled = None


def _build_program():
    nc = bacc.Bacc("TRN2", target_bir_lowering=False, debug=False,
                   num_devices=NCORES)
    f32 = mybir.dt.float32
    f16 = mybir.dt.float16

    xt_ap = nc.dram_tensor("xt", [P, NEC * NPAD], f16,
                           kind="ExternalInput").ap()
    ut_ap = nc.dram_tensor("ut", [P, NEC * NREL], f16,
                           kind="ExternalInput").ap()
    g_ap = nc.dram_tensor("g", [NPAIR, P, 2 * NREL], f16,
                          kind="ExternalOutput").ap()

    with tile.TileContext(nc) as tc:
        with tc.tile_pool(name="const", bufs=1) as cpool, \
             tc.tile_pool(name="xin", bufs=3) as xpool, \
             tc.tile_pool(name="zout", bufs=4) as zpool, \
             tc.tile_pool(name="z_ps", bufs=6, space="PSUM") as zppool:

            # folded weights UT[p, ec*500 + t] = U[t, ec*128+p]
            ut = cpool.tile([P, NEC * NREL], f16, tag="ut")
            nc.sync.dma_start(out=ut[:], in_=ut_ap[:, :])

            # stream X.T groups in; head/tail groups are one-shot tiles
            xts = []
            off = 0
            for gi, gn in enumerate(GROUPS):
                if gn == 1024:
                    xt_g = xpool.tile([P, NEC * gn], f16, tag="xt")
                else:
                    xt_g = cpool.tile([P, NEC * gn], f16, tag=f"xth{gi}")
                nc.sync.dma_start(out=xt_g[:],
                                  in_=xt_ap[:, NEC * off:NEC * (off + gn)])
                xts.append(xt_g)
                off += gn

            # output pairing: chunk 0 -> row 0 alone, then (1,2), (3,4), ...
            # (95,96), chunk 97 -> row 49 alone, so the final store is small
            # and issues immediately after the last copy.
            zsb = None
            chunk = 0
            off = 0
            for gi, gn in enumerate(GROUPS):
                xt_g = xts[gi]
                for cl in range(gn // P):
                    ps = zppool.tile([P, NREL], f32, tag="zps")
                    for ec in range(NEC):
                        nc.tensor.matmul(
                            out=ps[:],
                            lhsT=xt_g[:, ec * gn + cl * P:
                                      ec * gn + (cl + 1) * P],
                            rhs=ut[:, ec * NREL:(ec + 1) * NREL],
                            start=(ec == 0), stop=(ec == NEC - 1))
                    prow, pcol = (chunk + 1) // 2, ((chunk + 1) % 2) * NREL
                    if zsb is None:
                        zsb = zpool.tile([P, 2 * NREL], f16, tag="zsb")
                    dst = zsb[:, pcol:pcol + NREL]
                    if chunk % 2 == 0:
                        nc.scalar.copy(out=dst, in_=ps[:])
                    else:
                        nc.vector.tensor_copy(out=dst, in_=ps[:])
                    if pcol == 0 or chunk == NCHUNK - 1:
                        # row complete -> store (output DMAs ride the
                        # Activation HWDGE queue, never behind input loads)
                        width = NREL if chunk in (0, NCHUNK - 1) else 2 * NREL
                        nc.scalar.dma_start(out=g_ap[prow][:, :width],
                                            in_=zsb[:, :width])
                        zsb = None
                    chunk += 1
                off += gn

    nc.compile()
    return nc


def _prep_inputs(X_embed, edge_list_pred, edge_type_pred, W, ebvecs):
    """Shard/transpose inputs per core; build host-side gather indices."""
    X_embed = np.ascontiguousarray(X_embed, dtype=np.float32)
    W = np.asarray(W, dtype=np.float32)
    ebvecs = np.asarray(ebvecs, dtype=np.float32)

    # folded weights: U = ebvecs @ W  (500, 512); UT[p, ec*500+t]
    U = (ebvecs @ W).astype(np.float32)
    ut = np.ascontiguousarray(
        U.T.reshape(NEC, P, NREL).transpose(1, 0, 2).reshape(P, NEC * NREL)
    ).astype(np.float16)

    src = np.asarray(edge_list_pred[0], dtype=np.int64)
    tgt = np.asarray(edge_list_pred[1], dtype=np.int64)
    ty = np.asarray(edge_type_pred).reshape(-1).astype(np.int64)

    nodes = np.concatenate([src, tgt])                 # 600000
    types = np.concatenate([ty, ty])
    edges = np.concatenate([np.arange(E), np.arange(E)])
    signs = np.concatenate([np.ones(E, np.float64), -np.ones(E, np.float64)])

    owner = nodes // NPC                               # 0..7
    nloc = nodes - owner * NPC
    chunk = nloc >> 7
    part = nloc & (P - 1)
    prow = (chunk + 1) >> 1
    col = ((chunk + 1) & 1) * NREL + types

    # per-group column layout: [128, sum_g(4 * gn)] with group-local
    # embed-major blocks: cols(group g) = 4*off_g + ec*gn + j
    in_maps = []
    pick = []  # per core: (prow, partition, col, edge ids, signs)
    for i in range(NCORES):
        sel = owner == i
        xi = np.zeros((NPAD, EMBED), dtype=np.float32)
        xi[:NPC] = X_embed[i * NPC:(i + 1) * NPC]
        parts = []
        off = 0
        for gn in GROUPS:
            blk = xi[off:off + gn]                     # [gn, 512]
            parts.append(blk.reshape(gn, NEC, P).transpose(2, 1, 0)
                         .reshape(P, NEC * gn))
            off += gn
        xt = np.ascontiguousarray(np.concatenate(parts, axis=1)
                                  ).astype(np.float16)
        in_maps.append({"xt": xt, "ut": ut})
        pick.append((prow[sel], part[sel], col[sel], edges[sel], signs[sel]))
    return in_maps, pick


def kernel(X_embed, edge_list_pred, edge_type_pred, W, ebvecs,
           _trace=False, _tmpdir=None):
    global _compiled
    if _compiled is None:
        _compiled = _build_program()
    nc = _compiled

    in_maps, pick = _prep_inputs(X_embed, edge_list_pred, edge_type_pred,
                                 W, ebvecs)
    kw = {}
    if _trace:
        kw = {"trace": True, "tmpdir": _tmpdir}
    res = run_bass_kernel_spmd(nc, in_maps, list(range(NCORES)), **kw)

    scores = np.zeros(E, dtype=np.float64)
    for i in range(NCORES):
        gg, pp, col, ed, sg = pick[i]
        vals = res.results[i]["g"][gg, pp, col].astype(np.float64)
        scores += np.bincount(ed, weights=sg * vals, minlength=E)
    out = scores.astype(np.float32).reshape(1, E)
    if _trace:
        kernel.last_exec_time_ns = res.exec_time_ns
        kernel.last_results = res
    return out
